# revision 27
# baseline (speedup 1.0000x reference)
"""Trainium2 Bass kernel for nn_MetaLearningWithMemory.

Data-parallel over the query batch across 8 cores; the support-write scan is
restructured as a strictly-lower-triangular softmax fixed point
    W = rowsoftmax(base + tril(G, -1) @ W),   G = S S^T / sqrt(F)
solved with a few Jacobi iterations (exact to fp32 noise in <=4; we run 6),
replicated on every core.  The whole pipeline runs "transposed" (feature dim
on partitions, batch on free) so biases are per-partition and only x needs an
on-chip transpose (PE transpose of bf16 tiles).
"""

from contextlib import ExitStack

import numpy as np
import ml_dtypes

import concourse.bass as bass
import concourse.mybir as mybir
import concourse.tile as tile
from concourse import bacc
from concourse.bass_utils import run_bass_kernel_spmd

D_IN = 2048
F = 512
M = 256
NS = 512
H = 8
DH = 64
NW = 5
B = 16384
NCORES = 8
BC = B // NCORES          # 2048 batch rows per core
NB = 512                  # batch chunk (free dim of main matmuls)
NCH = BC // NB            # 4 chunks
NITER = 5                 # Jacobi softmax passes (converges at 4)
XT_MODE = "pe"            # "dmat": xbar DMA-transpose of bf16-truncated x
                          # "pe":   cast-DMA + PE transpose
INV_SQRT_F = float(F) ** -0.5
INV_SQRT_DH = float(DH) ** -0.5

bf16 = mybir.dt.float16
f32 = mybir.dt.float32
f32r = mybir.dt.float32r
FT = mybir.ActivationFunctionType


def _bf(a):
    return np.asarray(a, dtype=np.float32).astype(np.float16)


def build(stage="full", with_xs=True, repeat=1, opts=None):
    """stage: empty|dma|trans|feat|qf|attn|full|scan -- prefixes of the
    pipeline for perf bisection; "full" is the real kernel."""
    opts = dict(opts or {})
    O = lambda k, d: opts.get(k, d)
    ORDER = ["empty", "dma", "trans", "feat", "qf", "attn", "full"]
    lvl = ORDER.index(stage) if stage in ORDER else 0
    do_scan = stage in ("scan", "attn", "full")

    nc = bacc.Bacc("TRN2", target_bir_lowering=False)

    # ---- per-core external inputs (host-prepped layouts) ----
    xs = None
    if with_xs:
        if XT_MODE == "dmat":
            xs = nc.dram_tensor("xsb", [BC, D_IN, 2], bf16, kind="ExternalInput")
        else:
            xs = nc.dram_tensor("xs", [BC, D_IN], f32, kind="ExternalInput")
    wenc = nc.dram_tensor("wenc", [128, 16, F], bf16, kind="ExternalInput")
    sxt = nc.dram_tensor("sxt", [128, 16, NS], bf16, kind="ExternalInput")
    wq = nc.dram_tensor("wq", [128, 4, F], bf16, kind="ExternalInput")
    wclst = nc.dram_tensor("wclst", [128, 4, NW], bf16, kind="ExternalInput")
    wclsb = nc.dram_tensor("wclsb", [64, 8, NW], bf16, kind="ExternalInput")
    mkt0 = nc.dram_tensor("mkt0", [128, 4, M], f32, kind="ExternalInput")
    mkt0b = nc.dram_tensor("mkt0b", [128, 4, M], bf16, kind="ExternalInput")
    mvals = nc.dram_tensor("mvals", [128, 2, F], bf16, kind="ExternalInput")
    valsb = nc.dram_tensor("valsb", [128, 4, NW], bf16, kind="ExternalInput")
    benc = nc.dram_tensor("benc", [128, 4], f32, kind="ExternalInput")
    bq = nc.dram_tensor("bq", [128, 4], f32, kind="ExternalInput")
    bcls = nc.dram_tensor("bcls", [NW, 1], f32, kind="ExternalInput")
    y = nc.dram_tensor("y", [BC, NW], f32, kind="ExternalOutput")

    # ---- inline constants ----
    eye128 = nc.inline_tensor(np.eye(128, dtype=np.float16), name="eye128")
    eye5 = nc.inline_tensor(np.eye(NW, dtype=np.float32), name="eye5")
    # mask_su[s, t] = 1 if s < t  (strict upper; G[s,t] kept for s<t)
    mask_np = np.triu(np.ones((128, 128), np.float32), 1).astype(np.float16)
    mask_su = nc.inline_tensor(mask_np, name="mask_su")

    with tile.TileContext(nc) as tc:
        with ExitStack() as ctx:
            ep = ctx.enter_context
            const = ep(tc.tile_pool(name="const", bufs=1))
            persist = ep(tc.tile_pool(name="persist", bufs=1))
            xb_pool = ep(tc.tile_pool(name="xb", bufs=O("xb", 6)))
            xt_pool = ep(tc.tile_pool(
                name="xt", bufs=(2 if XT_MODE == "dmat" else O("xt", 12))))
            feat_pool = ep(tc.tile_pool(name="featT", bufs=O("featT", 8)))
            qf_pool = ep(tc.tile_pool(name="qfT", bufs=O("qfT", 8)))
            u_pool = ep(tc.tile_pool(name="u", bufs=O("u", 6)))
            nm_pool = ep(tc.tile_pool(name="nm", bufs=O("nm", 4)))
            rs_pool = ep(tc.tile_pool(name="rs", bufs=4))
            mo_pool = ep(tc.tile_pool(name="mo", bufs=O("mo", 16)))
            w_pool = ep(tc.tile_pool(name="w", bufs=2))
            usb_pool = ep(tc.tile_pool(name="usb", bufs=4))
            lg_pool = ep(tc.tile_pool(name="lg", bufs=2))
            y_pool = ep(tc.tile_pool(name="ysb", bufs=2))
            psA = ep(tc.tile_pool(name="psA", bufs=O("psA", 3), space="PSUM"))
            psS = ep(tc.tile_pool(name="psS", bufs=O("psS", 2), space="PSUM"))
            psT = ep(tc.tile_pool(name="psT", bufs=O("psT", 2), space="PSUM"))
            psL = ep(tc.tile_pool(name="psL", bufs=O("psL", 1), space="PSUM"))

            # ================= constant loads =================
            wenc_sb = const.tile([128, 16, F], bf16)
            nc.sync.dma_start(wenc_sb[:], wenc[:])
            sxt_sb = const.tile([128, 16, NS], bf16)
            nc.sync.dma_start(sxt_sb[:], sxt[:])
            wq_sb = const.tile([128, 4, F], bf16)
            nc.sync.dma_start(wq_sb[:], wq[:])
            wclst_sb = const.tile([128, 4, NW], bf16)
            nc.sync.dma_start(wclst_sb[:], wclst[:])
            wclsb_sb = const.tile([64, 8, NW], bf16)
            nc.sync.dma_start(wclsb_sb[:], wclsb[:])
            mkt0_sb = const.tile([128, 4, M], f32)
            nc.sync.dma_start(mkt0_sb[:], mkt0[:])
            mkt0b_sb = const.tile([128, 4, M], bf16)
            nc.sync.dma_start(mkt0b_sb[:], mkt0b[:])
            mv_sb = const.tile([128, 2, F], bf16)
            nc.sync.dma_start(mv_sb[:], mvals[:])
            vals_sb = const.tile([128, 4, NW], bf16)
            nc.sync.dma_start(vals_sb[:], valsb[:])
            benc_sb = const.tile([128, 4], f32)
            nc.sync.dma_start(benc_sb[:], benc[:])
            bq_sb = const.tile([128, 4], f32)
            nc.sync.dma_start(bq_sb[:], bq[:])
            bcls_sb = const.tile([NW, 1], f32)
            nc.sync.dma_start(bcls_sb[:], bcls[:])
            eye128_sb = const.tile([128, 128], bf16)
            nc.sync.dma_start(eye128_sb[:], eye128[:])
            eye5_sb = const.tile([NW, NW], f32)
            nc.sync.dma_start(eye5_sb[:], eye5[:])
            mask_sb = const.tile([128, 128], bf16)
            nc.sync.dma_start(mask_sb[:], mask_su[:])

            mkt_bf = persist.tile([128, 4, M], bf16, name="mkt_bf")
            mv_nat = persist.tile([128, 2, F], bf16, name="mv_nat")

            # ================= scan =================
            if do_scan:
                # S^T [f, t]: lhsT = W_enc k-tiles, rhs = sxT; + b_enc.
                st_bf = persist.tile([128, 4, NS], bf16, name="st_bf")
                sts_bf = persist.tile([128, 4, NS], bf16, name="sts_bf")
                for ft in range(4):
                    ps = psA.tile([128, NB], f32, tag="psA")
                    for j in range(16):
                        nc.tensor.matmul(
                            ps[:], wenc_sb[:, j, ft * 128:(ft + 1) * 128],
                            sxt_sb[:, j, :], start=(j == 0), stop=(j == 15),
                        )
                    nc.scalar.activation(st_bf[:, ft, :], ps[:], FT.Identity,
                                         bias=benc_sb[:, ft:ft + 1])
                    nc.scalar.activation(sts_bf[:, ft, :], ps[:], FT.Copy,
                                         scale=INV_SQRT_F)

                # S natural [t, f] via PE transpose of S^T
                s_sb = persist.tile([128, 4, F], bf16, name="s_sb")
                for tt in range(4):
                    pt = psT.tile([128, NB], bf16, tag="psT")
                    for ft in range(4):
                        nc.tensor.transpose(
                            pt[:, ft * 128:(ft + 1) * 128],
                            st_bf[:, ft, tt * 128:(tt + 1) * 128], eye128_sb[:])
                    nc.vector.tensor_copy(s_sb[:, tt, :], pt[:])

                # G[s, t] = (S S^T)/sqrt(F); diag blocks masked strict-upper.
                g_sb = persist.tile([128, 4, NS], bf16, name="g_sb")
                for ks in range(4):
                    ps = psA.tile([128, NB], f32, tag="psA")
                    for kf in range(4):
                        nc.tensor.matmul(
                            ps[:], st_bf[:, kf, ks * 128:(ks + 1) * 128],
                            sts_bf[:, kf, :], start=(kf == 0), stop=(kf == 3),
                        )
                    for tt in range(4):
                        dst = g_sb[:, ks, tt * 128:(tt + 1) * 128]
                        src = ps[:, tt * 128:(tt + 1) * 128]
                        if tt == ks:
                            nc.vector.tensor_mul(dst, src, mask_sb[:])
                        elif tt > ks:
                            nc.vector.tensor_copy(dst, src)

                # base[t, m] = S @ mem_keys^T / sqrt(F)
                base_sb = persist.tile([128, 4, M], f32, name="base_sb")
                for tt in range(4):
                    pl = psL.tile([128, M], f32, tag="psL")
                    for kf in range(4):
                        nc.tensor.matmul(
                            pl[:], sts_bf[:, kf, tt * 128:(tt + 1) * 128],
                            mkt0b_sb[:, kf, :], start=(kf == 0), stop=(kf == 3),
                        )
                    nc.scalar.copy(base_sb[:, tt, :], pl[:])

                # Jacobi iterations
                w_cur = w_pool.tile([128, 4, M], bf16, tag="wt", name="w_it0")
                for tt in range(4):
                    u = usb_pool.tile([128, M], f32, tag="usb")
                    ssum = rs_pool.tile([128, 1], f32, tag="scol")
                    nc.scalar.activation(u[:], base_sb[:, tt, :], FT.Exp,
                                         accum_out=ssum[:])
                    rcol = rs_pool.tile([128, 1], f32, tag="scol")
                    nc.vector.reciprocal(rcol[:], ssum[:])
                    nc.vector.tensor_scalar_mul(w_cur[:, tt, :], u[:], rcol[:])

                for it in range(1, O("niter", NITER)):
                    w_new = w_pool.tile([128, 4, M], bf16, tag="wt",
                                        name=f"w_it{it}")
                    for tt in range(4):
                        pl = psL.tile([128, M], f32, tag="psL")
                        for ks in range(tt + 1):
                            nc.tensor.matmul(
                                pl[:], g_sb[:, ks, tt * 128:(tt + 1) * 128],
                                w_cur[:, ks, :], start=(ks == 0), stop=(ks == tt),
                            )
                        nc.vector.tensor_add(pl[:], pl[:], base_sb[:, tt, :])
                        u = usb_pool.tile([128, M], f32, tag="usb")
                        ssum = rs_pool.tile([128, 1], f32, tag="scol")
                        nc.scalar.activation(u[:], pl[:], FT.Exp,
                                             accum_out=ssum[:])
                        rcol = rs_pool.tile([128, 1], f32, tag="scol")
                        nc.vector.reciprocal(rcol[:], ssum[:])
                        nc.vector.tensor_scalar_mul(w_new[:, tt, :], u[:], rcol[:])
                    w_cur = w_new

                # mk^T [f, m] += S-contraction of W
                for ft in range(4):
                    pl = psL.tile([128, M], f32, tag="psL")
                    for tt in range(4):
                        nc.tensor.matmul(
                            pl[:], s_sb[:, tt, ft * 128:(ft + 1) * 128],
                            w_cur[:, tt, :], start=(tt == 0), stop=(tt == 3),
                        )
                    nc.vector.tensor_add(mkt_bf[:, ft, :], pl[:],
                                         mkt0_sb[:, ft, :])

                # mv augmented [m, (h, d|1)]: values + ones col; one-hot update
                for mt in range(2):
                    nc.vector.tensor_copy(mv_nat[:, mt, NW:], mv_sb[:, mt, NW:])
                    pl = psL.tile([128, M], f32, tag="psL")
                    for tt in range(4):
                        nc.tensor.matmul(
                            pl[:, 0:NW], w_cur[:, tt, mt * 128:(mt + 1) * 128],
                            vals_sb[:, tt, :], start=(tt == 0), stop=(tt == 3),
                        )
                    nc.vector.tensor_add(mv_nat[:, mt, 0:NW], pl[:, 0:NW],
                                         mv_sb[:, mt, 0:NW])

            # ================= main batch pipeline =================
            for ch in range(NCH * repeat):
                ch = ch % NCH
                if lvl < 1:
                    break
                if XT_MODE == "dmat":
                    # transpose the full (lo, hi) bf16-pair matrix; row
                    # 2k+1 of each partition-interleaved group is the
                    # truncated-bf16 value of input dim p*16+k
                    xt_all = xt_pool.tile([128, 32, NB], bf16, tag="xt")
                    for bt in range(4):
                        r0 = ch * NB + bt * 128
                        nc.sync.dma_start(
                            xt_all[:, :, bt * 128:(bt + 1) * 128],
                            xs[r0:r0 + 128, :, :], transpose=True)
                    xts = [xt_all[:, 2 * j + 1, :] for j in range(16)]
                    if lvl < 3:
                        continue
                else:
                    xbs = []
                    for bt in range(4):
                        xb = xb_pool.tile([128, D_IN], bf16, tag="xb")
                        r0 = ch * NB + bt * 128
                        nc.gpsimd.dma_start(xb[:], xs[r0:r0 + 128, :])
                        xbs.append(xb)
                    if lvl < 2:
                        continue
                    xts = []
                    for jp in range(8):
                        pt = psT.tile([128, 2 * NB], bf16, tag="psT")
                        for jl in range(2):
                            j = 2 * jp + jl
                            for bt in range(4):
                                nc.tensor.transpose(
                                    pt[:, jl * NB + bt * 128:
                                       jl * NB + (bt + 1) * 128],
                                    xbs[bt][:, j * 128:(j + 1) * 128],
                                    eye128_sb[:])
                        xt = xt_pool.tile([128, 2, NB], bf16, tag="xt")
                        nc.vector.tensor_copy(xt[:], pt[:])
                        xts.extend([xt[:, 0, :], xt[:, 1, :]])
                    if lvl < 3:
                        continue

                featT = []
                for ft in range(4):
                    ps = psA.tile([128, NB], f32, tag="psA")
                    for j in range(16):
                        nc.tensor.matmul(
                            ps[:], wenc_sb[:, j, ft * 128:(ft + 1) * 128],
                            xts[j], start=(j == 0), stop=(j == 15),
                        )
                    fT = feat_pool.tile([128, NB], bf16, tag="featT")
                    nc.vector.tensor_scalar_add(fT[:], ps[:],
                                                benc_sb[:, ft:ft + 1])
                    featT.append(fT)
                if lvl < 4:
                    continue

                qfT = []
                for ft in range(4):
                    ps = psA.tile([128, NB], f32, tag="psA")
                    for k in range(4):
                        nc.tensor.matmul(
                            ps[:], wq_sb[:, k, ft * 128:(ft + 1) * 128],
                            featT[k][:], start=(k == 0), stop=(k == 3),
                        )
                    qT = qf_pool.tile([128, NB], bf16, tag="qfT")
                    nc.vector.tensor_scalar_add(qT[:], ps[:],
                                                bq_sb[:, ft:ft + 1])
                    qfT.append(qT)
                if lvl < 5:
                    continue

                # attention, row-wise: scores[b, m] -> exp+rowsum ->
                # normalize -> PE-transpose -> value matmul
                mos = []
                for h in range(H):
                    p0 = 64 * (h % 2)
                    ats = []
                    for bt in range(4):
                        ps = psS.tile([128, M], f32, tag="psS")
                        nc.tensor.matmul(
                            ps[:],
                            qfT[h // 2][p0:p0 + 64, bt * 128:(bt + 1) * 128],
                            mkt_bf[:, h // 2, :][p0:p0 + 64, :],
                            start=True, stop=True,
                        )
                        u = u_pool.tile([128, M], bf16, tag="u")
                        ssum = rs_pool.tile([128, 1], f32, tag="scol")
                        nc.scalar.activation(u[:], ps[:], FT.Exp,
                                             scale=INV_SQRT_DH,
                                             accum_out=ssum[:])
                        rcol = rs_pool.tile([128, 1], f32, tag="scol")
                        nc.vector.reciprocal(rcol[:], ssum[:])
                        at = u_pool.tile([128, M], bf16, tag="at")
                        nc.vector.tensor_scalar_mul(at[:], u[:], rcol[:])
                        ats.append(at)
                    pt = psT.tile([128, 2 * NB], bf16, tag="psT")
                    for bt in range(4):
                        for mj in range(2):
                            nc.tensor.transpose(
                                pt[:, mj * NB + bt * 128:
                                   mj * NB + (bt + 1) * 128],
                                ats[bt][:, mj * 128:(mj + 1) * 128],
                                eye128_sb[:])
                    att = nm_pool.tile([128, 2, NB], bf16, tag="att")
                    if h % 2 == 0:
                        nc.scalar.copy(att[:], pt[:])
                    else:
                        nc.vector.tensor_copy(att[:], pt[:])
                    pm = psA.tile([128, NB], f32, tag="psA")
                    for mj in range(2):
                        nc.tensor.matmul(
                            pm[0:DH, :],
                            mv_nat[:, mj, h * DH:(h + 1) * DH],
                            att[:, mj, :], start=(mj == 0), stop=(mj == 1),
                        )
                    mo = mo_pool.tile([DH, NB], bf16, tag="mo")
                    nc.scalar.copy(mo[:], pm[0:DH, :])
                    mos.append(mo)
                if lvl < 6:
                    continue

                # classifier: logits^T[c, b], then transpose + store
                ps = psA.tile([128, NB], f32, tag="psA")
                for k in range(4):
                    nc.tensor.matmul(
                        ps[0:NW, :], wclst_sb[:, k, :], featT[k][:],
                        start=(k == 0), stop=False,
                    )
                for h in range(H):
                    nc.tensor.matmul(
                        ps[0:NW, :], wclsb_sb[:, h, :], mos[h][:],
                        start=False, stop=(h == H - 1),
                    )
                lg = lg_pool.tile([NW, NB], f32, tag="lg")
                nc.scalar.activation(lg[:], ps[0:NW, :], FT.Identity,
                                     bias=bcls_sb[:])
                po = psL.tile([128, 4 * NW], f32, tag="psL")
                for jb in range(4):
                    nc.tensor.transpose(
                        po[:, jb * NW:(jb + 1) * NW],
                        lg[:, jb * 128:(jb + 1) * 128], eye5_sb[:])
                ysb = y_pool.tile([128, 4 * NW], f32, tag="ysb")
                nc.vector.tensor_copy(ysb[:], po[:])
                nc.sync.dma_start(
                    y[ch * NB:(ch + 1) * NB, :].rearrange(
                        "(jb p) c -> p jb c", p=128),
                    ysb[:].rearrange("p (jb c) -> p jb c", c=NW))

            if stage != "full":
                # truncated build: still produce the output tensor
                for ch in range(NCH):
                    ysb = y_pool.tile([128, 4 * NW], f32, tag="ysb")
                    nc.vector.memset(ysb[:], 0.0)
                    nc.sync.dma_start(
                        y[ch * NB:(ch + 1) * NB, :].rearrange(
                            "(jb p) c -> p jb c", p=128),
                        ysb[:].rearrange("p (jb c) -> p jb c", c=NW))

    nc.compile()
    return nc


def prep_inputs(inputs):
    """Host-side shard/layout prep. Returns per-core in_maps."""
    x = np.ascontiguousarray(np.asarray(inputs["x"], dtype=np.float32))
    sx = np.asarray(inputs["support_x"], dtype=np.float32)
    sy = np.asarray(inputs["support_y"]).astype(np.int64)
    W_enc = np.asarray(inputs["W_enc"], dtype=np.float32)
    b_enc = np.asarray(inputs["b_enc"], dtype=np.float32)
    W_q = np.asarray(inputs["W_q"], dtype=np.float32)
    b_q = np.asarray(inputs["b_q"], dtype=np.float32)
    W_cls = np.asarray(inputs["W_cls"], dtype=np.float32)
    b_cls = np.asarray(inputs["b_cls"], dtype=np.float32)
    mem_keys = np.asarray(inputs["mem_keys"], dtype=np.float32)
    mem_values = np.asarray(inputs["mem_values"], dtype=np.float32)

    def pk(a, p=128):  # [K, N] -> [p, K/p, N] partition-major tiles
        k, n = a.shape
        return np.ascontiguousarray(a.reshape(k // p, p, n).transpose(1, 0, 2))

    if XT_MODE == "dmat":
        # row-permuted tiles: partition p of k-tile holds input-dim p*16+k,
        # matching the xbar-transpose output interleave
        wenc_h = np.ascontiguousarray(_bf(W_enc).reshape(128, 16, F))
        sxt_h = np.ascontiguousarray(_bf(sx.T).reshape(128, 16, NS))
    else:
        wenc_h = pk(_bf(W_enc))                  # [128, 16, F]
        sxt_h = pk(_bf(sx.T))                    # [128, 16, NS]
    wq_h = pk(_bf(W_q))                          # [128, 4, F]
    wclst_h = pk(_bf(W_cls[:F]))                 # [128, 4, NW]
    wclsb_h = np.ascontiguousarray(
        _bf(W_cls[F:]).reshape(H, DH, NW).transpose(1, 0, 2))  # [64, 8, NW]
    mkt = np.ascontiguousarray(mem_keys.T)       # [F, M]
    mkt0_h = pk(mkt)
    mkt0b_h = pk(_bf(mkt))
    mvals_h = pk(_bf(mem_values))                # [128, 2, F]
    vals = np.zeros((NS, NW), np.float32)
    vals[np.arange(NS), sy] = 1.0
    valsb_h = pk(_bf(vals))                      # [128, 4, NW]
    benc_h = np.ascontiguousarray(b_enc.reshape(4, 128).T)
    bq_h = np.ascontiguousarray(b_q.reshape(4, 128).T)
    bcls_h = np.ascontiguousarray(b_cls.reshape(NW, 1))

    shared = dict(
        wenc=wenc_h, sxt=sxt_h, wq=wq_h, wclst=wclst_h, wclsb=wclsb_h,
        mkt0=mkt0_h, mkt0b=mkt0b_h, mvals=mvals_h, valsb=valsb_h,
        benc=benc_h, bq=bq_h, bcls=bcls_h,
    )
    in_maps = []
    for c in range(NCORES):
        m = dict(shared)
        if XT_MODE == "dmat":
            m["xsb"] = x[c * BC:(c + 1) * BC].view(
                ml_dtypes.bfloat16).reshape(BC, D_IN, 2)
        else:
            m["xs"] = x[c * BC:(c + 1) * BC]
        in_maps.append(m)
    return in_maps


def kernel_ex(inputs, trace=False, **kwargs):
    nc = build()
    in_maps = prep_inputs(inputs)
    res = run_bass_kernel_spmd(nc, in_maps, core_ids=list(range(NCORES)),
                               trace=trace, **kwargs)
    out = np.concatenate([r["y"] for r in res.results], axis=0)
    return out.astype(np.float32), res


def kernel(**inputs):
    out, _ = kernel_ex(inputs)
    return out


# revision 28
# speedup vs baseline: 1.2697x; 1.2697x over previous
"""Trainium2 Bass kernel for nn_MetaLearningWithMemory.

Data-parallel over the query batch across 8 cores; the support-write scan is
restructured as a strictly-lower-triangular softmax fixed point
    W = rowsoftmax(base + tril(G, -1) @ W),   G = S S^T / sqrt(F)
solved with a few Jacobi iterations (exact to fp32 noise in <=4; we run 6),
replicated on every core.  The whole pipeline runs "transposed" (feature dim
on partitions, batch on free) so biases are per-partition and only x needs an
on-chip transpose (PE transpose of bf16 tiles).
"""

from contextlib import ExitStack

import numpy as np
import ml_dtypes

import concourse.bass as bass
import concourse.mybir as mybir
import concourse.tile as tile
from concourse import bacc
from concourse.bass_utils import run_bass_kernel_spmd

D_IN = 2048
F = 512
M = 256
NS = 512
H = 8
DH = 64
NW = 5
B = 16384
NCORES = 8
BC = B // NCORES          # 2048 batch rows per core
NB = 512                  # batch chunk (free dim of main matmuls)
NCH = BC // NB            # 4 chunks
NITER = 5                 # Jacobi softmax passes (converges at 4)
XT_MODE = "pe"            # "dmat": xbar DMA-transpose of bf16-truncated x
                          # "pe":   cast-DMA + PE transpose
INV_SQRT_F = float(F) ** -0.5
INV_SQRT_DH = float(DH) ** -0.5

bf16 = mybir.dt.float16
f32 = mybir.dt.float32
f32r = mybir.dt.float32r
FT = mybir.ActivationFunctionType


def _bf(a):
    return np.asarray(a, dtype=np.float32).astype(np.float16)


def build(stage="full", with_xs=True, repeat=1, opts=None):
    """stage: empty|dma|trans|feat|qf|attn|full|scan -- prefixes of the
    pipeline for perf bisection; "full" is the real kernel."""
    opts = dict(opts or {})
    O = lambda k, d: opts.get(k, d)
    ORDER = ["empty", "dma", "trans", "feat", "qf", "attn", "full"]
    lvl = ORDER.index(stage) if stage in ORDER else 0
    do_scan = stage in ("scan", "attn", "full")

    nc = bacc.Bacc("TRN2", target_bir_lowering=False)

    # ---- per-core external inputs (host-prepped layouts) ----
    xs = None
    if with_xs:
        if XT_MODE == "dmat":
            xs = nc.dram_tensor("xsb", [BC, D_IN, 2], bf16, kind="ExternalInput")
        else:
            xs = nc.dram_tensor("xs", [BC, D_IN], f32, kind="ExternalInput")
    wenc = nc.dram_tensor("wenc", [128, 16, F], bf16, kind="ExternalInput")
    sxt = nc.dram_tensor("sxt", [128, 16, NS], bf16, kind="ExternalInput")
    wq = nc.dram_tensor("wq", [128, 4, F], bf16, kind="ExternalInput")
    wclst = nc.dram_tensor("wclst", [128, 4, NW], bf16, kind="ExternalInput")
    wclsb = nc.dram_tensor("wclsb", [128, 4, NW], bf16, kind="ExternalInput")
    mkt0 = nc.dram_tensor("mkt0", [128, 4, M], f32, kind="ExternalInput")
    mkt0b = nc.dram_tensor("mkt0b", [128, 4, M], bf16, kind="ExternalInput")
    mvals = nc.dram_tensor("mvals", [128, 2, F], bf16, kind="ExternalInput")
    valsb = nc.dram_tensor("valsb", [128, 4, NW], bf16, kind="ExternalInput")
    benc = nc.dram_tensor("benc", [128, 4], f32, kind="ExternalInput")
    bq = nc.dram_tensor("bq", [128, 4], f32, kind="ExternalInput")
    bcls = nc.dram_tensor("bcls", [NW, 1], f32, kind="ExternalInput")
    y = nc.dram_tensor("y", [BC, NW], f32, kind="ExternalOutput")

    # ---- inline constants ----
    eye128 = nc.inline_tensor(np.eye(128, dtype=np.float16), name="eye128")
    eye5 = nc.inline_tensor(np.eye(NW, dtype=np.float32), name="eye5")
    # mask_su[s, t] = 1 if s < t  (strict upper; G[s,t] kept for s<t)
    mask_np = np.triu(np.ones((128, 128), np.float32), 1).astype(np.float16)
    mask_su = nc.inline_tensor(mask_np, name="mask_su")

    with tile.TileContext(nc) as tc:
        with ExitStack() as ctx:
            ep = ctx.enter_context
            const = ep(tc.tile_pool(name="const", bufs=1))
            persist = ep(tc.tile_pool(name="persist", bufs=1))
            xb_pool = ep(tc.tile_pool(name="xb", bufs=O("xb", 6)))
            xt_pool = ep(tc.tile_pool(
                name="xt", bufs=(2 if XT_MODE == "dmat" else O("xt", 12))))
            feat_pool = ep(tc.tile_pool(name="featT", bufs=O("featT", 8)))
            qf_pool = ep(tc.tile_pool(name="qfT", bufs=O("qfT", 8)))
            u_pool = ep(tc.tile_pool(name="u", bufs=O("u", 6)))
            nm_pool = ep(tc.tile_pool(name="nm", bufs=O("nm", 4)))
            rs_pool = ep(tc.tile_pool(name="rs", bufs=4))
            mo_pool = ep(tc.tile_pool(name="mo", bufs=O("mo", 8)))
            w_pool = ep(tc.tile_pool(name="w", bufs=2))
            usb_pool = ep(tc.tile_pool(name="usb", bufs=4))
            lg_pool = ep(tc.tile_pool(name="lg", bufs=2))
            y_pool = ep(tc.tile_pool(name="ysb", bufs=2))
            psA = ep(tc.tile_pool(name="psA", bufs=O("psA", 3), space="PSUM"))
            psS = ep(tc.tile_pool(name="psS", bufs=O("psS", 2), space="PSUM"))
            psT = ep(tc.tile_pool(name="psT", bufs=O("psT", 2), space="PSUM"))
            psL = ep(tc.tile_pool(name="psL", bufs=O("psL", 1), space="PSUM"))

            # ================= constant loads =================
            wenc_sb = const.tile([128, 16, F], bf16)
            nc.sync.dma_start(wenc_sb[:], wenc[:])
            sxt_sb = const.tile([128, 16, NS], bf16)
            nc.sync.dma_start(sxt_sb[:], sxt[:])
            wq_sb = const.tile([128, 4, F], bf16)
            nc.sync.dma_start(wq_sb[:], wq[:])
            wclst_sb = const.tile([128, 4, NW], bf16)
            nc.sync.dma_start(wclst_sb[:], wclst[:])
            wclsb_sb = const.tile([128, 4, NW], bf16)
            nc.sync.dma_start(wclsb_sb[:], wclsb[:])
            mkt0_sb = const.tile([128, 4, M], f32)
            nc.sync.dma_start(mkt0_sb[:], mkt0[:])
            mkt0b_sb = const.tile([128, 4, M], bf16)
            nc.sync.dma_start(mkt0b_sb[:], mkt0b[:])
            mv_sb = const.tile([128, 2, F], bf16)
            nc.sync.dma_start(mv_sb[:], mvals[:])
            vals_sb = const.tile([128, 4, NW], bf16)
            nc.sync.dma_start(vals_sb[:], valsb[:])
            benc_sb = const.tile([128, 4], f32)
            nc.sync.dma_start(benc_sb[:], benc[:])
            bq_sb = const.tile([128, 4], f32)
            nc.sync.dma_start(bq_sb[:], bq[:])
            bcls_sb = const.tile([NW, 1], f32)
            nc.sync.dma_start(bcls_sb[:], bcls[:])
            eye128_sb = const.tile([128, 128], bf16)
            nc.sync.dma_start(eye128_sb[:], eye128[:])
            eye5_sb = const.tile([NW, NW], f32)
            nc.sync.dma_start(eye5_sb[:], eye5[:])
            mask_sb = const.tile([128, 128], bf16)
            nc.sync.dma_start(mask_sb[:], mask_su[:])

            mkt_bf = persist.tile([128, 4, M], bf16, name="mkt_bf")
            mv_nat = persist.tile([128, 2, F], bf16, name="mv_nat")

            # ================= scan =================
            if do_scan:
                # S^T [f, t]: lhsT = W_enc k-tiles, rhs = sxT; + b_enc.
                st_bf = persist.tile([128, 4, NS], bf16, name="st_bf")
                sts_bf = persist.tile([128, 4, NS], bf16, name="sts_bf")
                for ft in range(4):
                    ps = psA.tile([128, NB], f32, tag="psA")
                    for j in range(16):
                        nc.tensor.matmul(
                            ps[:], wenc_sb[:, j, ft * 128:(ft + 1) * 128],
                            sxt_sb[:, j, :], start=(j == 0), stop=(j == 15),
                        )
                    nc.scalar.activation(st_bf[:, ft, :], ps[:], FT.Identity,
                                         bias=benc_sb[:, ft:ft + 1])
                    nc.scalar.activation(sts_bf[:, ft, :], ps[:], FT.Copy,
                                         scale=INV_SQRT_F)

                # S natural [t, f] via PE transpose of S^T
                s_sb = persist.tile([128, 4, F], bf16, name="s_sb")
                for tt in range(4):
                    pt = psT.tile([128, NB], bf16, tag="psT")
                    for ft in range(4):
                        nc.tensor.transpose(
                            pt[:, ft * 128:(ft + 1) * 128],
                            st_bf[:, ft, tt * 128:(tt + 1) * 128], eye128_sb[:])
                    nc.vector.tensor_copy(s_sb[:, tt, :], pt[:])

                # G[s, t] = (S S^T)/sqrt(F); diag blocks masked strict-upper.
                g_sb = persist.tile([128, 4, NS], bf16, name="g_sb")
                for ks in range(4):
                    ps = psA.tile([128, NB], f32, tag="psA")
                    for kf in range(4):
                        nc.tensor.matmul(
                            ps[:], st_bf[:, kf, ks * 128:(ks + 1) * 128],
                            sts_bf[:, kf, :], start=(kf == 0), stop=(kf == 3),
                        )
                    for tt in range(4):
                        dst = g_sb[:, ks, tt * 128:(tt + 1) * 128]
                        src = ps[:, tt * 128:(tt + 1) * 128]
                        if tt == ks:
                            nc.vector.tensor_mul(dst, src, mask_sb[:])
                        elif tt > ks:
                            nc.vector.tensor_copy(dst, src)

                # base[t, m] = S @ mem_keys^T / sqrt(F)
                base_sb = persist.tile([128, 4, M], f32, name="base_sb")
                for tt in range(4):
                    pl = psL.tile([128, M], f32, tag="psL")
                    for kf in range(4):
                        nc.tensor.matmul(
                            pl[:], sts_bf[:, kf, tt * 128:(tt + 1) * 128],
                            mkt0b_sb[:, kf, :], start=(kf == 0), stop=(kf == 3),
                        )
                    nc.scalar.copy(base_sb[:, tt, :], pl[:])

                # Jacobi iterations
                w_cur = w_pool.tile([128, 4, M], bf16, tag="wt", name="w_it0")
                for tt in range(4):
                    u = usb_pool.tile([128, M], f32, tag="usb")
                    ssum = rs_pool.tile([128, 1], f32, tag="scol")
                    nc.scalar.activation(u[:], base_sb[:, tt, :], FT.Exp,
                                         accum_out=ssum[:])
                    rcol = rs_pool.tile([128, 1], f32, tag="scol")
                    nc.vector.reciprocal(rcol[:], ssum[:])
                    nc.vector.tensor_scalar_mul(w_cur[:, tt, :], u[:], rcol[:])

                for it in range(1, O("niter", NITER)):
                    w_new = w_pool.tile([128, 4, M], bf16, tag="wt",
                                        name=f"w_it{it}")
                    for tt in range(4):
                        pl = psL.tile([128, M], f32, tag="psL")
                        for ks in range(tt + 1):
                            nc.tensor.matmul(
                                pl[:], g_sb[:, ks, tt * 128:(tt + 1) * 128],
                                w_cur[:, ks, :], start=(ks == 0), stop=(ks == tt),
                            )
                        nc.vector.tensor_add(pl[:], pl[:], base_sb[:, tt, :])
                        u = usb_pool.tile([128, M], f32, tag="usb")
                        ssum = rs_pool.tile([128, 1], f32, tag="scol")
                        nc.scalar.activation(u[:], pl[:], FT.Exp,
                                             accum_out=ssum[:])
                        rcol = rs_pool.tile([128, 1], f32, tag="scol")
                        nc.vector.reciprocal(rcol[:], ssum[:])
                        nc.vector.tensor_scalar_mul(w_new[:, tt, :], u[:], rcol[:])
                    w_cur = w_new

                # mk^T [f, m] += S-contraction of W
                for ft in range(4):
                    pl = psL.tile([128, M], f32, tag="psL")
                    for tt in range(4):
                        nc.tensor.matmul(
                            pl[:], s_sb[:, tt, ft * 128:(ft + 1) * 128],
                            w_cur[:, tt, :], start=(tt == 0), stop=(tt == 3),
                        )
                    nc.vector.tensor_add(mkt_bf[:, ft, :], pl[:],
                                         mkt0_sb[:, ft, :])

                # mv augmented [m, (h, d|1)]: values + ones col; one-hot update
                for mt in range(2):
                    nc.vector.tensor_copy(mv_nat[:, mt, NW:], mv_sb[:, mt, NW:])
                    pl = psL.tile([128, M], f32, tag="psL")
                    for tt in range(4):
                        nc.tensor.matmul(
                            pl[:, 0:NW], w_cur[:, tt, mt * 128:(mt + 1) * 128],
                            vals_sb[:, tt, :], start=(tt == 0), stop=(tt == 3),
                        )
                    nc.vector.tensor_add(mv_nat[:, mt, 0:NW], pl[:, 0:NW],
                                         mv_sb[:, mt, 0:NW])

            # ================= main batch pipeline =================
            for ch in range(NCH * repeat):
                ch = ch % NCH
                if lvl < 1:
                    break
                if XT_MODE == "dmat":
                    # transpose the full (lo, hi) bf16-pair matrix; row
                    # 2k+1 of each partition-interleaved group is the
                    # truncated-bf16 value of input dim p*16+k
                    xt_all = xt_pool.tile([128, 32, NB], bf16, tag="xt")
                    for bt in range(4):
                        r0 = ch * NB + bt * 128
                        nc.sync.dma_start(
                            xt_all[:, :, bt * 128:(bt + 1) * 128],
                            xs[r0:r0 + 128, :, :], transpose=True)
                    xts = [xt_all[:, 2 * j + 1, :] for j in range(16)]
                    if lvl < 3:
                        continue
                else:
                    xbs = []
                    for bt in range(4):
                        xb = xb_pool.tile([128, D_IN], bf16, tag="xb")
                        r0 = ch * NB + bt * 128
                        nc.gpsimd.dma_start(xb[:], xs[r0:r0 + 128, :])
                        xbs.append(xb)
                    if lvl < 2:
                        continue
                    xts = []
                    for jp in range(8):
                        pt = psT.tile([128, 2 * NB], bf16, tag="psT")
                        for jl in range(2):
                            j = 2 * jp + jl
                            for bt in range(4):
                                nc.tensor.transpose(
                                    pt[:, jl * NB + bt * 128:
                                       jl * NB + (bt + 1) * 128],
                                    xbs[bt][:, j * 128:(j + 1) * 128],
                                    eye128_sb[:])
                        xt = xt_pool.tile([128, 2, NB], bf16, tag="xt")
                        nc.vector.tensor_copy(xt[:], pt[:])
                        xts.extend([xt[:, 0, :], xt[:, 1, :]])
                    if lvl < 3:
                        continue

                featT = []
                for ft in range(4):
                    ps = psA.tile([128, NB], f32, tag="psA")
                    for j in range(16):
                        nc.tensor.matmul(
                            ps[:], wenc_sb[:, j, ft * 128:(ft + 1) * 128],
                            xts[j], start=(j == 0), stop=(j == 15),
                        )
                    fT = feat_pool.tile([128, NB], bf16, tag="featT")
                    nc.vector.tensor_scalar_add(fT[:], ps[:],
                                                benc_sb[:, ft:ft + 1])
                    featT.append(fT)
                if lvl < 4:
                    continue

                qfT = []
                for ft in range(4):
                    ps = psA.tile([128, NB], f32, tag="psA")
                    for k in range(4):
                        nc.tensor.matmul(
                            ps[:], wq_sb[:, k, ft * 128:(ft + 1) * 128],
                            featT[k][:], start=(k == 0), stop=(k == 3),
                        )
                    qT = qf_pool.tile([128, NB], bf16, tag="qfT")
                    nc.vector.tensor_scalar_add(qT[:], ps[:],
                                                bq_sb[:, ft:ft + 1])
                    qfT.append(qT)
                if lvl < 5:
                    continue

                # attention, row-wise: scores[b, m] -> exp+rowsum ->
                # normalize -> PE-transpose -> value matmul
                mos = []
                for h in range(H):
                    p0 = 64 * (h % 2)
                    ats = []
                    for bt in range(4):
                        ps = psS.tile([128, M], f32, tag="psS")
                        nc.tensor.matmul(
                            ps[:],
                            qfT[h // 2][p0:p0 + 64, bt * 128:(bt + 1) * 128],
                            mkt_bf[:, h // 2, :][p0:p0 + 64, :],
                            start=True, stop=True,
                        )
                        u = u_pool.tile([128, M], bf16, tag="u")
                        ssum = rs_pool.tile([128, 1], f32, tag="scol")
                        nc.scalar.activation(u[:], ps[:], FT.Exp,
                                             scale=INV_SQRT_DH,
                                             accum_out=ssum[:])
                        rcol = rs_pool.tile([128, 1], f32, tag="scol")
                        nc.vector.reciprocal(rcol[:], ssum[:])
                        at = u_pool.tile([128, M], bf16, tag="at")
                        nc.vector.tensor_scalar_mul(at[:], u[:], rcol[:])
                        ats.append(at)
                    pt = psT.tile([128, 2 * NB], bf16, tag="psT")
                    for bt in range(4):
                        for mj in range(2):
                            nc.tensor.transpose(
                                pt[:, mj * NB + bt * 128:
                                   mj * NB + (bt + 1) * 128],
                                ats[bt][:, mj * 128:(mj + 1) * 128],
                                eye128_sb[:])
                    att = nm_pool.tile([128, 2, NB], bf16, tag="att")
                    if h % 2 == 0:
                        nc.scalar.copy(att[:], pt[:])
                    else:
                        nc.vector.tensor_copy(att[:], pt[:])
                    if h % 2 == 0:
                        pm = psA.tile([128, NB], f32, tag="psA")
                        pm_cur = pm
                    p0o = DH * (h % 2)
                    for mj in range(2):
                        nc.tensor.matmul(
                            pm_cur[p0o:p0o + DH, :],
                            mv_nat[:, mj, h * DH:(h + 1) * DH],
                            att[:, mj, :], start=(mj == 0), stop=(mj == 1),
                        )
                    if h % 2 == 1:
                        mo = mo_pool.tile([128, NB], bf16, tag="mo")
                        nc.scalar.copy(mo[:], pm_cur[:])
                        mos.append(mo)
                if lvl < 6:
                    continue

                # classifier: logits^T[c, b], then transpose + store
                ps = psA.tile([128, NB], f32, tag="psA")
                for k in range(4):
                    nc.tensor.matmul(
                        ps[0:NW, :], wclst_sb[:, k, :], featT[k][:],
                        start=(k == 0), stop=False,
                    )
                for hp in range(4):
                    nc.tensor.matmul(
                        ps[0:NW, :], wclsb_sb[:, hp, :], mos[hp][:],
                        start=False, stop=(hp == 3),
                    )
                lg = lg_pool.tile([NW, NB], f32, tag="lg")
                nc.scalar.activation(lg[:], ps[0:NW, :], FT.Identity,
                                     bias=bcls_sb[:])
                po = psL.tile([128, 4 * NW], f32, tag="psL")
                for jb in range(4):
                    nc.tensor.transpose(
                        po[:, jb * NW:(jb + 1) * NW],
                        lg[:, jb * 128:(jb + 1) * 128], eye5_sb[:])
                ysb = y_pool.tile([128, 4 * NW], f32, tag="ysb")
                nc.vector.tensor_copy(ysb[:], po[:])
                nc.sync.dma_start(
                    y[ch * NB:(ch + 1) * NB, :].rearrange(
                        "(jb p) c -> p jb c", p=128),
                    ysb[:].rearrange("p (jb c) -> p jb c", c=NW))

            if stage != "full":
                # truncated build: still produce the output tensor
                for ch in range(NCH):
                    ysb = y_pool.tile([128, 4 * NW], f32, tag="ysb")
                    nc.vector.memset(ysb[:], 0.0)
                    nc.sync.dma_start(
                        y[ch * NB:(ch + 1) * NB, :].rearrange(
                            "(jb p) c -> p jb c", p=128),
                        ysb[:].rearrange("p (jb c) -> p jb c", c=NW))

    nc.compile()
    return nc


def prep_inputs(inputs):
    """Host-side shard/layout prep. Returns per-core in_maps."""
    x = np.ascontiguousarray(np.asarray(inputs["x"], dtype=np.float32))
    sx = np.asarray(inputs["support_x"], dtype=np.float32)
    sy = np.asarray(inputs["support_y"]).astype(np.int64)
    W_enc = np.asarray(inputs["W_enc"], dtype=np.float32)
    b_enc = np.asarray(inputs["b_enc"], dtype=np.float32)
    W_q = np.asarray(inputs["W_q"], dtype=np.float32)
    b_q = np.asarray(inputs["b_q"], dtype=np.float32)
    W_cls = np.asarray(inputs["W_cls"], dtype=np.float32)
    b_cls = np.asarray(inputs["b_cls"], dtype=np.float32)
    mem_keys = np.asarray(inputs["mem_keys"], dtype=np.float32)
    mem_values = np.asarray(inputs["mem_values"], dtype=np.float32)

    def pk(a, p=128):  # [K, N] -> [p, K/p, N] partition-major tiles
        k, n = a.shape
        return np.ascontiguousarray(a.reshape(k // p, p, n).transpose(1, 0, 2))

    if XT_MODE == "dmat":
        # row-permuted tiles: partition p of k-tile holds input-dim p*16+k,
        # matching the xbar-transpose output interleave
        wenc_h = np.ascontiguousarray(_bf(W_enc).reshape(128, 16, F))
        sxt_h = np.ascontiguousarray(_bf(sx.T).reshape(128, 16, NS))
    else:
        wenc_h = pk(_bf(W_enc))                  # [128, 16, F]
        sxt_h = pk(_bf(sx.T))                    # [128, 16, NS]
    wq_h = pk(_bf(W_q))                          # [128, 4, F]
    wclst_h = pk(_bf(W_cls[:F]))                 # [128, 4, NW]
    wclsb_h = pk(_bf(W_cls[F:]))                 # [128, 4, NW] head-pairs
    mkt = np.ascontiguousarray(mem_keys.T)       # [F, M]
    mkt0_h = pk(mkt)
    mkt0b_h = pk(_bf(mkt))
    mvals_h = pk(_bf(mem_values))                # [128, 2, F]
    vals = np.zeros((NS, NW), np.float32)
    vals[np.arange(NS), sy] = 1.0
    valsb_h = pk(_bf(vals))                      # [128, 4, NW]
    benc_h = np.ascontiguousarray(b_enc.reshape(4, 128).T)
    bq_h = np.ascontiguousarray(b_q.reshape(4, 128).T)
    bcls_h = np.ascontiguousarray(b_cls.reshape(NW, 1))

    shared = dict(
        wenc=wenc_h, sxt=sxt_h, wq=wq_h, wclst=wclst_h, wclsb=wclsb_h,
        mkt0=mkt0_h, mkt0b=mkt0b_h, mvals=mvals_h, valsb=valsb_h,
        benc=benc_h, bq=bq_h, bcls=bcls_h,
    )
    in_maps = []
    for c in range(NCORES):
        m = dict(shared)
        if XT_MODE == "dmat":
            m["xsb"] = x[c * BC:(c + 1) * BC].view(
                ml_dtypes.bfloat16).reshape(BC, D_IN, 2)
        else:
            m["xs"] = x[c * BC:(c + 1) * BC]
        in_maps.append(m)
    return in_maps


def kernel_ex(inputs, trace=False, **kwargs):
    nc = build()
    in_maps = prep_inputs(inputs)
    res = run_bass_kernel_spmd(nc, in_maps, core_ids=list(range(NCORES)),
                               trace=trace, **kwargs)
    out = np.concatenate([r["y"] for r in res.results], axis=0)
    return out.astype(np.float32), res


def kernel(**inputs):
    out, _ = kernel_ex(inputs)
    return out


# revision 29
# speedup vs baseline: 1.2917x; 1.0173x over previous
"""Trainium2 Bass kernel for nn_MetaLearningWithMemory.

Data-parallel over the query batch across 8 cores; the support-write scan is
restructured as a strictly-lower-triangular softmax fixed point
    W = rowsoftmax(base + tril(G, -1) @ W),   G = S S^T / sqrt(F)
solved with a few Jacobi iterations (exact to fp32 noise in <=4; we run 6),
replicated on every core.  The whole pipeline runs "transposed" (feature dim
on partitions, batch on free) so biases are per-partition and only x needs an
on-chip transpose (PE transpose of bf16 tiles).
"""

from contextlib import ExitStack

import numpy as np
import ml_dtypes

import concourse.bass as bass
import concourse.mybir as mybir
import concourse.tile as tile
from concourse import bacc
from concourse.bass_utils import run_bass_kernel_spmd

D_IN = 2048
F = 512
M = 256
NS = 512
H = 8
DH = 64
NW = 5
B = 16384
NCORES = 8
BC = B // NCORES          # 2048 batch rows per core
NB = 512                  # batch chunk (free dim of main matmuls)
NCH = BC // NB            # 4 chunks
NITER = 5                 # Jacobi softmax passes (converges at 4)
XT_MODE = "pe"            # "dmat": xbar DMA-transpose of bf16-truncated x
                          # "pe":   cast-DMA + PE transpose
INV_SQRT_F = float(F) ** -0.5
INV_SQRT_DH = float(DH) ** -0.5

bf16 = mybir.dt.float16
f32 = mybir.dt.float32
f32r = mybir.dt.float32r
FT = mybir.ActivationFunctionType


def _bf(a):
    return np.asarray(a, dtype=np.float32).astype(np.float16)


def build(stage="full", with_xs=True, repeat=1, opts=None):
    """stage: empty|dma|trans|feat|qf|attn|full|scan -- prefixes of the
    pipeline for perf bisection; "full" is the real kernel."""
    opts = dict(opts or {})
    O = lambda k, d: opts.get(k, d)
    ORDER = ["empty", "dma", "trans", "feat", "qf", "attn", "full"]
    lvl = ORDER.index(stage) if stage in ORDER else 0
    do_scan = stage in ("scan", "attn", "full")

    nc = bacc.Bacc("TRN2", target_bir_lowering=False)

    # ---- per-core external inputs (host-prepped layouts) ----
    xs = None
    if with_xs:
        if XT_MODE == "dmat":
            xs = nc.dram_tensor("xsb", [BC, D_IN, 2], bf16, kind="ExternalInput")
        else:
            xs = nc.dram_tensor("xs", [BC, D_IN], f32, kind="ExternalInput")
    wenc = nc.dram_tensor("wenc", [128, 16, F], bf16, kind="ExternalInput")
    sxt = nc.dram_tensor("sxt", [128, 16, NS], bf16, kind="ExternalInput")
    wq = nc.dram_tensor("wq", [128, 4, F], bf16, kind="ExternalInput")
    wclst = nc.dram_tensor("wclst", [128, 4, NW], bf16, kind="ExternalInput")
    wclsb = nc.dram_tensor("wclsb", [128, 4, NW], bf16, kind="ExternalInput")
    mkt0 = nc.dram_tensor("mkt0", [128, 4, M], f32, kind="ExternalInput")
    mkt0b = nc.dram_tensor("mkt0b", [128, 4, M], bf16, kind="ExternalInput")
    mvals = nc.dram_tensor("mvals", [128, 2, F], bf16, kind="ExternalInput")
    valsb = nc.dram_tensor("valsb", [128, 4, NW], bf16, kind="ExternalInput")
    benc = nc.dram_tensor("benc", [128, 4], f32, kind="ExternalInput")
    bq = nc.dram_tensor("bq", [128, 4], f32, kind="ExternalInput")
    bcls = nc.dram_tensor("bcls", [NW, 1], f32, kind="ExternalInput")
    y = nc.dram_tensor("y", [BC, NW], f32, kind="ExternalOutput")

    # ---- inline constants ----
    eye128 = nc.inline_tensor(np.eye(128, dtype=np.float16), name="eye128")
    eye5 = nc.inline_tensor(np.eye(NW, dtype=np.float32), name="eye5")
    # mask_su[s, t] = 1 if s < t  (strict upper; G[s,t] kept for s<t)
    mask_np = np.triu(np.full((128, 128), INV_SQRT_F, np.float32), 1).astype(np.float16)
    mask_su = nc.inline_tensor(mask_np, name="mask_su")

    with tile.TileContext(nc) as tc:
        with ExitStack() as ctx:
            ep = ctx.enter_context
            const = ep(tc.tile_pool(name="const", bufs=1))
            persist = ep(tc.tile_pool(name="persist", bufs=1))
            xb_pool = ep(tc.tile_pool(name="xb", bufs=O("xb", 6)))
            xt_pool = ep(tc.tile_pool(
                name="xt", bufs=(2 if XT_MODE == "dmat" else O("xt", 12))))
            feat_pool = ep(tc.tile_pool(name="featT", bufs=O("featT", 8)))
            qf_pool = ep(tc.tile_pool(name="qfT", bufs=O("qfT", 8)))
            u_pool = ep(tc.tile_pool(name="u", bufs=O("u", 6)))
            nm_pool = ep(tc.tile_pool(name="nm", bufs=O("nm", 4)))
            rs_pool = ep(tc.tile_pool(name="rs", bufs=4))
            mo_pool = ep(tc.tile_pool(name="mo", bufs=O("mo", 8)))
            w_pool = ep(tc.tile_pool(name="w", bufs=2))
            usb_pool = ep(tc.tile_pool(name="usb", bufs=4))
            lg_pool = ep(tc.tile_pool(name="lg", bufs=2))
            y_pool = ep(tc.tile_pool(name="ysb", bufs=2))
            psA = ep(tc.tile_pool(name="psA", bufs=O("psA", 3), space="PSUM"))
            psS = ep(tc.tile_pool(name="psS", bufs=O("psS", 2), space="PSUM"))
            psT = ep(tc.tile_pool(name="psT", bufs=O("psT", 2), space="PSUM"))
            psL = ep(tc.tile_pool(name="psL", bufs=O("psL", 1), space="PSUM"))

            # ================= constant loads =================
            wenc_sb = const.tile([128, 16, F], bf16)
            nc.sync.dma_start(wenc_sb[:], wenc[:])
            sxt_sb = const.tile([128, 16, NS], bf16)
            nc.sync.dma_start(sxt_sb[:], sxt[:])
            wq_sb = const.tile([128, 4, F], bf16)
            nc.sync.dma_start(wq_sb[:], wq[:])
            wclst_sb = const.tile([128, 4, NW], bf16)
            nc.sync.dma_start(wclst_sb[:], wclst[:])
            wclsb_sb = const.tile([128, 4, NW], bf16)
            nc.sync.dma_start(wclsb_sb[:], wclsb[:])
            mkt0_sb = const.tile([128, 4, M], f32)
            nc.sync.dma_start(mkt0_sb[:], mkt0[:])
            mkt0b_sb = const.tile([128, 4, M], bf16)
            nc.sync.dma_start(mkt0b_sb[:], mkt0b[:])
            mv_sb = const.tile([128, 2, F], bf16)
            nc.sync.dma_start(mv_sb[:], mvals[:])
            vals_sb = const.tile([128, 4, NW], bf16)
            nc.sync.dma_start(vals_sb[:], valsb[:])
            benc_sb = const.tile([128, 4], f32)
            nc.sync.dma_start(benc_sb[:], benc[:])
            bq_sb = const.tile([128, 4], f32)
            nc.sync.dma_start(bq_sb[:], bq[:])
            bcls_sb = const.tile([NW, 1], f32)
            nc.sync.dma_start(bcls_sb[:], bcls[:])
            eye128_sb = const.tile([128, 128], bf16)
            nc.sync.dma_start(eye128_sb[:], eye128[:])
            eye5_sb = const.tile([NW, NW], f32)
            nc.sync.dma_start(eye5_sb[:], eye5[:])
            mask_sb = const.tile([128, 128], bf16)
            nc.sync.dma_start(mask_sb[:], mask_su[:])

            mkt_bf = persist.tile([128, 4, M], bf16, name="mkt_bf")
            mv_nat = persist.tile([128, 2, F], bf16, name="mv_nat")

            # ================= scan =================
            if do_scan:
                # S^T [f, t]: lhsT = W_enc k-tiles, rhs = sxT; + b_enc.
                st_bf = persist.tile([128, 4, NS], bf16, name="st_bf")
                for ft in range(4):
                    ps = psA.tile([128, NB], f32, tag="psA")
                    for j in range(16):
                        nc.tensor.matmul(
                            ps[:], wenc_sb[:, j, ft * 128:(ft + 1) * 128],
                            sxt_sb[:, j, :], start=(j == 0), stop=(j == 15),
                        )
                    nc.scalar.activation(st_bf[:, ft, :], ps[:], FT.Identity,
                                         bias=benc_sb[:, ft:ft + 1])

                # S natural [t, f] via PE transpose of S^T
                s_sb = persist.tile([128, 4, F], bf16, name="s_sb")
                for tt in range(4):
                    pt = psT.tile([128, NB], bf16, tag="psT")
                    for ft in range(4):
                        nc.tensor.transpose(
                            pt[:, ft * 128:(ft + 1) * 128],
                            st_bf[:, ft, tt * 128:(tt + 1) * 128], eye128_sb[:])
                    nc.vector.tensor_copy(s_sb[:, tt, :], pt[:])

                # G[s, t] = (S S^T)/sqrt(F); diag blocks masked strict-upper.
                g_sb = persist.tile([128, 4, NS], bf16, name="g_sb")
                for ks in range(4):
                    ps = psA.tile([128, NB], f32, tag="psA")
                    for kf in range(4):
                        nc.tensor.matmul(
                            ps[:], st_bf[:, kf, ks * 128:(ks + 1) * 128],
                            st_bf[:, kf, :], start=(kf == 0), stop=(kf == 3),
                        )
                    for tt in range(4):
                        dst = g_sb[:, ks, tt * 128:(tt + 1) * 128]
                        src = ps[:, tt * 128:(tt + 1) * 128]
                        if tt == ks:
                            nc.vector.tensor_mul(dst, src, mask_sb[:])
                        elif tt > ks:
                            nc.vector.tensor_scalar_mul(dst, src, INV_SQRT_F)

                # base[t, m] = S @ mem_keys^T / sqrt(F)
                base_sb = persist.tile([128, 4, M], f32, name="base_sb")
                for tt in range(4):
                    pl = psL.tile([128, M], f32, tag="psL")
                    for kf in range(4):
                        nc.tensor.matmul(
                            pl[:], st_bf[:, kf, tt * 128:(tt + 1) * 128],
                            mkt0b_sb[:, kf, :], start=(kf == 0), stop=(kf == 3),
                        )
                    nc.scalar.copy(base_sb[:, tt, :], pl[:])

                # Jacobi iterations
                w_cur = w_pool.tile([128, 4, M], bf16, tag="wt", name="w_it0")
                for tt in range(4):
                    u = usb_pool.tile([128, M], f32, tag="usb")
                    ssum = rs_pool.tile([128, 1], f32, tag="scol")
                    nc.scalar.activation(u[:], base_sb[:, tt, :], FT.Exp,
                                         accum_out=ssum[:])
                    rcol = rs_pool.tile([128, 1], f32, tag="scol")
                    nc.vector.reciprocal(rcol[:], ssum[:])
                    nc.vector.tensor_scalar_mul(w_cur[:, tt, :], u[:], rcol[:])

                for it in range(1, O("niter", NITER)):
                    w_new = w_pool.tile([128, 4, M], bf16, tag="wt",
                                        name=f"w_it{it}")
                    for tt in range(4):
                        pl = psL.tile([128, M], f32, tag="psL")
                        for ks in range(tt + 1):
                            nc.tensor.matmul(
                                pl[:], g_sb[:, ks, tt * 128:(tt + 1) * 128],
                                w_cur[:, ks, :], start=(ks == 0), stop=(ks == tt),
                            )
                        nc.vector.tensor_add(pl[:], pl[:], base_sb[:, tt, :])
                        u = usb_pool.tile([128, M], f32, tag="usb")
                        ssum = rs_pool.tile([128, 1], f32, tag="scol")
                        nc.scalar.activation(u[:], pl[:], FT.Exp,
                                             accum_out=ssum[:])
                        rcol = rs_pool.tile([128, 1], f32, tag="scol")
                        nc.vector.reciprocal(rcol[:], ssum[:])
                        nc.vector.tensor_scalar_mul(w_new[:, tt, :], u[:], rcol[:])
                    w_cur = w_new

                # mk^T [f, m] += S-contraction of W
                for ft in range(4):
                    pl = psL.tile([128, M], f32, tag="psL")
                    for tt in range(4):
                        nc.tensor.matmul(
                            pl[:], s_sb[:, tt, ft * 128:(ft + 1) * 128],
                            w_cur[:, tt, :], start=(tt == 0), stop=(tt == 3),
                        )
                    nc.vector.tensor_add(mkt_bf[:, ft, :], pl[:],
                                         mkt0_sb[:, ft, :])

                # mv augmented [m, (h, d|1)]: values + ones col; one-hot update
                for mt in range(2):
                    nc.vector.tensor_copy(mv_nat[:, mt, NW:], mv_sb[:, mt, NW:])
                    pl = psL.tile([128, M], f32, tag="psL")
                    for tt in range(4):
                        nc.tensor.matmul(
                            pl[:, 0:NW], w_cur[:, tt, mt * 128:(mt + 1) * 128],
                            vals_sb[:, tt, :], start=(tt == 0), stop=(tt == 3),
                        )
                    nc.vector.tensor_add(mv_nat[:, mt, 0:NW], pl[:, 0:NW],
                                         mv_sb[:, mt, 0:NW])

            # ================= main batch pipeline =================
            for ch in range(NCH * repeat):
                ch = ch % NCH
                if lvl < 1:
                    break
                if XT_MODE == "dmat":
                    # transpose the full (lo, hi) bf16-pair matrix; row
                    # 2k+1 of each partition-interleaved group is the
                    # truncated-bf16 value of input dim p*16+k
                    xt_all = xt_pool.tile([128, 32, NB], bf16, tag="xt")
                    for bt in range(4):
                        r0 = ch * NB + bt * 128
                        nc.sync.dma_start(
                            xt_all[:, :, bt * 128:(bt + 1) * 128],
                            xs[r0:r0 + 128, :, :], transpose=True)
                    xts = [xt_all[:, 2 * j + 1, :] for j in range(16)]
                    if lvl < 3:
                        continue
                else:
                    xbs = []
                    for bt in range(4):
                        xb = xb_pool.tile([128, D_IN], bf16, tag="xb")
                        r0 = ch * NB + bt * 128
                        nc.gpsimd.dma_start(xb[:], xs[r0:r0 + 128, :])
                        xbs.append(xb)
                    if lvl < 2:
                        continue
                    xts = []
                    for jp in range(8):
                        pt = psT.tile([128, 2 * NB], bf16, tag="psT")
                        for jl in range(2):
                            j = 2 * jp + jl
                            for bt in range(4):
                                nc.tensor.transpose(
                                    pt[:, jl * NB + bt * 128:
                                       jl * NB + (bt + 1) * 128],
                                    xbs[bt][:, j * 128:(j + 1) * 128],
                                    eye128_sb[:])
                        xt = xt_pool.tile([128, 2, NB], bf16, tag="xt")
                        nc.vector.tensor_copy(xt[:], pt[:])
                        xts.extend([xt[:, 0, :], xt[:, 1, :]])
                    if lvl < 3:
                        continue

                featT = []
                for ft in range(4):
                    ps = psA.tile([128, NB], f32, tag="psA")
                    for j in range(16):
                        nc.tensor.matmul(
                            ps[:], wenc_sb[:, j, ft * 128:(ft + 1) * 128],
                            xts[j], start=(j == 0), stop=(j == 15),
                        )
                    fT = feat_pool.tile([128, NB], bf16, tag="featT")
                    nc.vector.tensor_scalar_add(fT[:], ps[:],
                                                benc_sb[:, ft:ft + 1])
                    featT.append(fT)
                if lvl < 4:
                    continue

                qfT = []
                for ft in range(4):
                    ps = psA.tile([128, NB], f32, tag="psA")
                    for k in range(4):
                        nc.tensor.matmul(
                            ps[:], wq_sb[:, k, ft * 128:(ft + 1) * 128],
                            featT[k][:], start=(k == 0), stop=(k == 3),
                        )
                    qT = qf_pool.tile([128, NB], bf16, tag="qfT")
                    nc.vector.tensor_scalar_add(qT[:], ps[:],
                                                bq_sb[:, ft:ft + 1])
                    qfT.append(qT)
                if lvl < 5:
                    continue

                # attention, row-wise: scores[b, m] -> exp+rowsum ->
                # normalize -> PE-transpose -> value matmul
                mos = []
                for h in range(H):
                    p0 = 64 * (h % 2)
                    ats = []
                    for bt in range(4):
                        ps = psS.tile([128, M], f32, tag="psS")
                        nc.tensor.matmul(
                            ps[:],
                            qfT[h // 2][p0:p0 + 64, bt * 128:(bt + 1) * 128],
                            mkt_bf[:, h // 2, :][p0:p0 + 64, :],
                            start=True, stop=True,
                        )
                        u = u_pool.tile([128, M], bf16, tag="u")
                        ssum = rs_pool.tile([128, 1], f32, tag="scol")
                        nc.scalar.activation(u[:], ps[:], FT.Exp,
                                             scale=INV_SQRT_DH,
                                             accum_out=ssum[:])
                        rcol = rs_pool.tile([128, 1], f32, tag="scol")
                        nc.vector.reciprocal(rcol[:], ssum[:])
                        at = u_pool.tile([128, M], bf16, tag="at")
                        nc.vector.tensor_scalar_mul(at[:], u[:], rcol[:])
                        ats.append(at)
                    pt = psT.tile([128, 2 * NB], bf16, tag="psT")
                    for bt in range(4):
                        for mj in range(2):
                            nc.tensor.transpose(
                                pt[:, mj * NB + bt * 128:
                                   mj * NB + (bt + 1) * 128],
                                ats[bt][:, mj * 128:(mj + 1) * 128],
                                eye128_sb[:])
                    att = nm_pool.tile([128, 2, NB], bf16, tag="att")
                    if h % 2 == 0:
                        nc.scalar.copy(att[:], pt[:])
                    else:
                        nc.vector.tensor_copy(att[:], pt[:])
                    if h % 2 == 0:
                        pm = psA.tile([128, NB], f32, tag="psA")
                        pm_cur = pm
                    p0o = DH * (h % 2)
                    for mj in range(2):
                        nc.tensor.matmul(
                            pm_cur[p0o:p0o + DH, :],
                            mv_nat[:, mj, h * DH:(h + 1) * DH],
                            att[:, mj, :], start=(mj == 0), stop=(mj == 1),
                        )
                    if h % 2 == 1:
                        mo = mo_pool.tile([128, NB], bf16, tag="mo")
                        nc.scalar.copy(mo[:], pm_cur[:])
                        mos.append(mo)
                if lvl < 6:
                    continue

                # classifier: logits^T[c, b], then transpose + store
                ps = psA.tile([128, NB], f32, tag="psA")
                for k in range(4):
                    nc.tensor.matmul(
                        ps[0:NW, :], wclst_sb[:, k, :], featT[k][:],
                        start=(k == 0), stop=False,
                    )
                for hp in range(4):
                    nc.tensor.matmul(
                        ps[0:NW, :], wclsb_sb[:, hp, :], mos[hp][:],
                        start=False, stop=(hp == 3),
                    )
                lg = lg_pool.tile([NW, NB], f32, tag="lg")
                nc.scalar.activation(lg[:], ps[0:NW, :], FT.Identity,
                                     bias=bcls_sb[:])
                po = psL.tile([128, 4 * NW], f32, tag="psL")
                for jb in range(4):
                    nc.tensor.transpose(
                        po[:, jb * NW:(jb + 1) * NW],
                        lg[:, jb * 128:(jb + 1) * 128], eye5_sb[:])
                ysb = y_pool.tile([128, 4 * NW], f32, tag="ysb")
                nc.vector.tensor_copy(ysb[:], po[:])
                nc.sync.dma_start(
                    y[ch * NB:(ch + 1) * NB, :].rearrange(
                        "(jb p) c -> p jb c", p=128),
                    ysb[:].rearrange("p (jb c) -> p jb c", c=NW))

            if stage != "full":
                # truncated build: still produce the output tensor
                for ch in range(NCH):
                    ysb = y_pool.tile([128, 4 * NW], f32, tag="ysb")
                    nc.vector.memset(ysb[:], 0.0)
                    nc.sync.dma_start(
                        y[ch * NB:(ch + 1) * NB, :].rearrange(
                            "(jb p) c -> p jb c", p=128),
                        ysb[:].rearrange("p (jb c) -> p jb c", c=NW))

    nc.compile()
    return nc


def prep_inputs(inputs):
    """Host-side shard/layout prep. Returns per-core in_maps."""
    x = np.ascontiguousarray(np.asarray(inputs["x"], dtype=np.float32))
    sx = np.asarray(inputs["support_x"], dtype=np.float32)
    sy = np.asarray(inputs["support_y"]).astype(np.int64)
    W_enc = np.asarray(inputs["W_enc"], dtype=np.float32)
    b_enc = np.asarray(inputs["b_enc"], dtype=np.float32)
    W_q = np.asarray(inputs["W_q"], dtype=np.float32)
    b_q = np.asarray(inputs["b_q"], dtype=np.float32)
    W_cls = np.asarray(inputs["W_cls"], dtype=np.float32)
    b_cls = np.asarray(inputs["b_cls"], dtype=np.float32)
    mem_keys = np.asarray(inputs["mem_keys"], dtype=np.float32)
    mem_values = np.asarray(inputs["mem_values"], dtype=np.float32)

    def pk(a, p=128):  # [K, N] -> [p, K/p, N] partition-major tiles
        k, n = a.shape
        return np.ascontiguousarray(a.reshape(k // p, p, n).transpose(1, 0, 2))

    if XT_MODE == "dmat":
        # row-permuted tiles: partition p of k-tile holds input-dim p*16+k,
        # matching the xbar-transpose output interleave
        wenc_h = np.ascontiguousarray(_bf(W_enc).reshape(128, 16, F))
        sxt_h = np.ascontiguousarray(_bf(sx.T).reshape(128, 16, NS))
    else:
        wenc_h = pk(_bf(W_enc))                  # [128, 16, F]
        sxt_h = pk(_bf(sx.T))                    # [128, 16, NS]
    wq_h = pk(_bf(W_q))                          # [128, 4, F]
    wclst_h = pk(_bf(W_cls[:F]))                 # [128, 4, NW]
    wclsb_h = pk(_bf(W_cls[F:]))                 # [128, 4, NW] head-pairs
    mkt = np.ascontiguousarray(mem_keys.T)       # [F, M]
    mkt0_h = pk(mkt)
    mkt0b_h = pk(_bf(mkt * (512.0 ** -0.5)))
    mvals_h = pk(_bf(mem_values))                # [128, 2, F]
    vals = np.zeros((NS, NW), np.float32)
    vals[np.arange(NS), sy] = 1.0
    valsb_h = pk(_bf(vals))                      # [128, 4, NW]
    benc_h = np.ascontiguousarray(b_enc.reshape(4, 128).T)
    bq_h = np.ascontiguousarray(b_q.reshape(4, 128).T)
    bcls_h = np.ascontiguousarray(b_cls.reshape(NW, 1))

    shared = dict(
        wenc=wenc_h, sxt=sxt_h, wq=wq_h, wclst=wclst_h, wclsb=wclsb_h,
        mkt0=mkt0_h, mkt0b=mkt0b_h, mvals=mvals_h, valsb=valsb_h,
        benc=benc_h, bq=bq_h, bcls=bcls_h,
    )
    in_maps = []
    for c in range(NCORES):
        m = dict(shared)
        if XT_MODE == "dmat":
            m["xsb"] = x[c * BC:(c + 1) * BC].view(
                ml_dtypes.bfloat16).reshape(BC, D_IN, 2)
        else:
            m["xs"] = x[c * BC:(c + 1) * BC]
        in_maps.append(m)
    return in_maps


def kernel_ex(inputs, trace=False, **kwargs):
    nc = build()
    in_maps = prep_inputs(inputs)
    res = run_bass_kernel_spmd(nc, in_maps, core_ids=list(range(NCORES)),
                               trace=trace, **kwargs)
    out = np.concatenate([r["y"] for r in res.results], axis=0)
    return out.astype(np.float32), res


def kernel(**inputs):
    out, _ = kernel_ex(inputs)
    return out


# revision 30
# speedup vs baseline: 1.3279x; 1.0280x over previous
"""Trainium2 Bass kernel for nn_MetaLearningWithMemory.

Data-parallel over the query batch across 8 cores; the support-write scan is
restructured as a strictly-lower-triangular softmax fixed point
    W = rowsoftmax(base + tril(G, -1) @ W),   G = S S^T / sqrt(F)
solved with a few Jacobi iterations (exact to fp32 noise in <=4; we run 5),
replicated on every core.  The whole pipeline runs "transposed" (feature dim
on partitions, batch on free) so biases are per-partition and only x needs an
on-chip transpose (PE transpose of fp16 tiles).  Attention is row-wise
(softmax sums via ACT accum_out), heads are paired in the value/classifier
stage, and all matmul datapaths are fp16 with fp32 PSUM accumulation.
"""

from contextlib import ExitStack

import numpy as np
import ml_dtypes

import concourse.bass as bass
import concourse.mybir as mybir
import concourse.tile as tile
from concourse import bacc
from concourse.bass_utils import run_bass_kernel_spmd

D_IN = 2048
F = 512
M = 256
NS = 512
H = 8
DH = 64
NW = 5
B = 16384
NCORES = 8
BC = B // NCORES          # 2048 batch rows per core
NB = 512                  # batch chunk (free dim of main matmuls)
NCH = BC // NB            # 4 chunks
NITER = 5                 # Jacobi softmax passes (converges at 4)
XT_MODE = "pe"            # "dmat": xbar DMA-transpose of bf16-truncated x
                          # "pe":   cast-DMA + PE transpose
INV_SQRT_F = float(F) ** -0.5
INV_SQRT_DH = float(DH) ** -0.5

bf16 = mybir.dt.float16
f32 = mybir.dt.float32
f32r = mybir.dt.float32r
FT = mybir.ActivationFunctionType


def _bf(a):
    return np.asarray(a, dtype=np.float32).astype(np.float16)


def build(stage="full", with_xs=True, repeat=1, opts=None):
    """stage: empty|dma|trans|feat|qf|attn|full|scan -- prefixes of the
    pipeline for perf bisection; "full" is the real kernel."""
    opts = dict(opts or {})
    O = lambda k, d: opts.get(k, d)
    ORDER = ["empty", "dma", "trans", "feat", "qf", "attn", "full"]
    lvl = ORDER.index(stage) if stage in ORDER else 0
    do_scan = stage in ("scan", "attn", "full")

    nc = bacc.Bacc("TRN2", target_bir_lowering=False)

    # ---- per-core external inputs (host-prepped layouts) ----
    xs = None
    if with_xs:
        if XT_MODE == "dmat":
            xs = nc.dram_tensor("xsb", [BC, D_IN, 2], bf16, kind="ExternalInput")
        else:
            xs = nc.dram_tensor("xs", [BC, D_IN], f32, kind="ExternalInput")
    wenc = nc.dram_tensor("wenc", [128, 16, F], bf16, kind="ExternalInput")
    sxt = nc.dram_tensor("sxt", [128, 16, NS], bf16, kind="ExternalInput")
    wq = nc.dram_tensor("wq", [128, 4, F], bf16, kind="ExternalInput")
    wclst = nc.dram_tensor("wclst", [128, 4, NW], bf16, kind="ExternalInput")
    wclsb = nc.dram_tensor("wclsb", [128, 4, NW], bf16, kind="ExternalInput")
    mkt0 = nc.dram_tensor("mkt0", [128, 4, M], f32, kind="ExternalInput")
    mkt0b = nc.dram_tensor("mkt0b", [128, 4, M], bf16, kind="ExternalInput")
    mvals = nc.dram_tensor("mvals", [128, 2, F], bf16, kind="ExternalInput")
    valsb = nc.dram_tensor("valsb", [128, 4, NW], bf16, kind="ExternalInput")
    benc = nc.dram_tensor("benc", [128, 4], f32, kind="ExternalInput")
    bq = nc.dram_tensor("bq", [128, 4], f32, kind="ExternalInput")
    bcls = nc.dram_tensor("bcls", [NW, 1], f32, kind="ExternalInput")
    y = nc.dram_tensor("y", [BC, NW], f32, kind="ExternalOutput")

    # ---- inline constants ----
    eye128 = nc.inline_tensor(np.eye(128, dtype=np.float16), name="eye128")
    eye5 = nc.inline_tensor(np.eye(NW, dtype=np.float32), name="eye5")
    # mask_su[s, t] = 1 if s < t  (strict upper; G[s,t] kept for s<t)
    mask_np = np.triu(np.full((128, 128), INV_SQRT_F, np.float32), 1).astype(np.float16)
    mask_su = nc.inline_tensor(mask_np, name="mask_su")

    with tile.TileContext(nc) as tc:
        with ExitStack() as ctx:
            ep = ctx.enter_context
            const = ep(tc.tile_pool(name="const", bufs=1))
            persist = ep(tc.tile_pool(name="persist", bufs=1))
            xb_pool = ep(tc.tile_pool(name="xb", bufs=O("xb", 6)))
            xt_pool = ep(tc.tile_pool(
                name="xt", bufs=(2 if XT_MODE == "dmat" else O("xt", 12))))
            feat_pool = ep(tc.tile_pool(name="featT", bufs=O("featT", 8)))
            qf_pool = ep(tc.tile_pool(name="qfT", bufs=O("qfT", 8)))
            u_pool = ep(tc.tile_pool(name="u", bufs=O("u", 6)))
            nm_pool = ep(tc.tile_pool(name="nm", bufs=O("nm", 4)))
            rs_pool = ep(tc.tile_pool(name="rs", bufs=4))
            mo_pool = ep(tc.tile_pool(name="mo", bufs=O("mo", 8)))
            w_pool = ep(tc.tile_pool(name="w", bufs=2))
            usb_pool = ep(tc.tile_pool(name="usb", bufs=4))
            lg_pool = ep(tc.tile_pool(name="lg", bufs=2))
            y_pool = ep(tc.tile_pool(name="ysb", bufs=2))
            psA = ep(tc.tile_pool(name="psA", bufs=O("psA", 3), space="PSUM"))
            psS = ep(tc.tile_pool(name="psS", bufs=O("psS", 2), space="PSUM"))
            psT = ep(tc.tile_pool(name="psT", bufs=O("psT", 2), space="PSUM"))
            psL = ep(tc.tile_pool(name="psL", bufs=O("psL", 1), space="PSUM"))

            # ================= constant loads =================
            wenc_sb = const.tile([128, 16, F], bf16)
            nc.sync.dma_start(wenc_sb[:], wenc[:])
            sxt_sb = const.tile([128, 16, NS], bf16)
            nc.sync.dma_start(sxt_sb[:], sxt[:])
            wq_sb = const.tile([128, 4, F], bf16)
            nc.sync.dma_start(wq_sb[:], wq[:])
            wclst_sb = const.tile([128, 4, NW], bf16)
            nc.sync.dma_start(wclst_sb[:], wclst[:])
            wclsb_sb = const.tile([128, 4, NW], bf16)
            nc.sync.dma_start(wclsb_sb[:], wclsb[:])
            mkt0_sb = const.tile([128, 4, M], f32)
            nc.sync.dma_start(mkt0_sb[:], mkt0[:])
            mkt0b_sb = const.tile([128, 4, M], bf16)
            nc.sync.dma_start(mkt0b_sb[:], mkt0b[:])
            mv_sb = const.tile([128, 2, F], bf16)
            nc.sync.dma_start(mv_sb[:], mvals[:])
            vals_sb = const.tile([128, 4, NW], bf16)
            nc.sync.dma_start(vals_sb[:], valsb[:])
            benc_sb = const.tile([128, 4], f32)
            nc.sync.dma_start(benc_sb[:], benc[:])
            bq_sb = const.tile([128, 4], f32)
            nc.sync.dma_start(bq_sb[:], bq[:])
            bcls_sb = const.tile([NW, 1], f32)
            nc.sync.dma_start(bcls_sb[:], bcls[:])
            eye128_sb = const.tile([128, 128], bf16)
            nc.sync.dma_start(eye128_sb[:], eye128[:])
            eye5_sb = const.tile([NW, NW], f32)
            nc.sync.dma_start(eye5_sb[:], eye5[:])
            mask_sb = const.tile([128, 128], bf16)
            nc.sync.dma_start(mask_sb[:], mask_su[:])

            mkt_bf = persist.tile([128, 4, M], bf16, name="mkt_bf")
            mv_nat = persist.tile([128, 2, F], bf16, name="mv_nat")

            # ================= scan =================
            if do_scan:
                # S^T [f, t]: lhsT = W_enc k-tiles, rhs = sxT; + b_enc.
                st_bf = persist.tile([128, 4, NS], bf16, name="st_bf")
                for ft in range(4):
                    ps = psA.tile([128, NB], f32, tag="psA")
                    for j in range(16):
                        nc.tensor.matmul(
                            ps[:], wenc_sb[:, j, ft * 128:(ft + 1) * 128],
                            sxt_sb[:, j, :], start=(j == 0), stop=(j == 15),
                        )
                    nc.scalar.activation(st_bf[:, ft, :], ps[:], FT.Identity,
                                         bias=benc_sb[:, ft:ft + 1])

                # S natural [t, f] via PE transpose of S^T
                s_sb = persist.tile([128, 4, F], bf16, name="s_sb")
                for tt in range(4):
                    pt = psT.tile([128, NB], bf16, tag="psT")
                    for ft in range(4):
                        nc.tensor.transpose(
                            pt[:, ft * 128:(ft + 1) * 128],
                            st_bf[:, ft, tt * 128:(tt + 1) * 128], eye128_sb[:])
                    nc.vector.tensor_copy(s_sb[:, tt, :], pt[:])

                # G[s, t] = (S S^T)/sqrt(F); diag blocks masked strict-upper.
                g_sb = persist.tile([128, 4, NS], bf16, name="g_sb")
                for ks in range(4):
                    ps = psA.tile([128, NB], f32, tag="psA")
                    for kf in range(4):
                        nc.tensor.matmul(
                            ps[:], st_bf[:, kf, ks * 128:(ks + 1) * 128],
                            st_bf[:, kf, :], start=(kf == 0), stop=(kf == 3),
                        )
                    for tt in range(4):
                        dst = g_sb[:, ks, tt * 128:(tt + 1) * 128]
                        src = ps[:, tt * 128:(tt + 1) * 128]
                        if tt == ks:
                            nc.vector.tensor_mul(dst, src, mask_sb[:])
                        elif tt > ks:
                            nc.vector.tensor_scalar_mul(dst, src, INV_SQRT_F)

                # base[t, m] = S @ mem_keys^T / sqrt(F)
                base_sb = persist.tile([128, 4, M], f32, name="base_sb")
                for tt in range(4):
                    pl = psL.tile([128, M], f32, tag="psL")
                    for kf in range(4):
                        nc.tensor.matmul(
                            pl[:], st_bf[:, kf, tt * 128:(tt + 1) * 128],
                            mkt0b_sb[:, kf, :], start=(kf == 0), stop=(kf == 3),
                        )
                    nc.scalar.copy(base_sb[:, tt, :], pl[:])

                # Jacobi iterations
                w_cur = w_pool.tile([128, 4, M], bf16, tag="wt", name="w_it0")
                for tt in range(4):
                    u = usb_pool.tile([128, M], f32, tag="usb")
                    ssum = rs_pool.tile([128, 1], f32, tag="scol")
                    nc.scalar.activation(u[:], base_sb[:, tt, :], FT.Exp,
                                         accum_out=ssum[:])
                    rcol = rs_pool.tile([128, 1], f32, tag="scol")
                    nc.vector.reciprocal(rcol[:], ssum[:])
                    nc.vector.tensor_scalar_mul(w_cur[:, tt, :], u[:], rcol[:])

                for it in range(1, O("niter", NITER)):
                    w_new = w_pool.tile([128, 4, M], bf16, tag="wt",
                                        name=f"w_it{it}")
                    for tt in range(4):
                        pl = psL.tile([128, M], f32, tag="psL")
                        for ks in range(tt + 1):
                            nc.tensor.matmul(
                                pl[:], g_sb[:, ks, tt * 128:(tt + 1) * 128],
                                w_cur[:, ks, :], start=(ks == 0), stop=(ks == tt),
                            )
                        nc.vector.tensor_add(pl[:], pl[:], base_sb[:, tt, :])
                        u = usb_pool.tile([128, M], f32, tag="usb")
                        ssum = rs_pool.tile([128, 1], f32, tag="scol")
                        nc.scalar.activation(u[:], pl[:], FT.Exp,
                                             accum_out=ssum[:])
                        rcol = rs_pool.tile([128, 1], f32, tag="scol")
                        nc.vector.reciprocal(rcol[:], ssum[:])
                        nc.vector.tensor_scalar_mul(w_new[:, tt, :], u[:], rcol[:])
                    w_cur = w_new

                # mk^T [f, m] += S-contraction of W
                for ft in range(4):
                    pl = psL.tile([128, M], f32, tag="psL")
                    for tt in range(4):
                        nc.tensor.matmul(
                            pl[:], s_sb[:, tt, ft * 128:(ft + 1) * 128],
                            w_cur[:, tt, :], start=(tt == 0), stop=(tt == 3),
                        )
                    nc.vector.tensor_add(mkt_bf[:, ft, :], pl[:],
                                         mkt0_sb[:, ft, :])

                # mv augmented [m, (h, d|1)]: values + ones col; one-hot update
                for mt in range(2):
                    nc.vector.tensor_copy(mv_nat[:, mt, NW:], mv_sb[:, mt, NW:])
                    pl = psL.tile([128, M], f32, tag="psL")
                    for tt in range(4):
                        nc.tensor.matmul(
                            pl[:, 0:NW], w_cur[:, tt, mt * 128:(mt + 1) * 128],
                            vals_sb[:, tt, :], start=(tt == 0), stop=(tt == 3),
                        )
                    nc.vector.tensor_add(mv_nat[:, mt, 0:NW], pl[:, 0:NW],
                                         mv_sb[:, mt, 0:NW])

            # ================= main batch pipeline =================
            for ch in range(NCH * repeat):
                ch = ch % NCH
                if lvl < 1:
                    break
                if XT_MODE == "dmat":
                    # transpose the full (lo, hi) bf16-pair matrix; row
                    # 2k+1 of each partition-interleaved group is the
                    # truncated-bf16 value of input dim p*16+k
                    xt_all = xt_pool.tile([128, 32, NB], bf16, tag="xt")
                    for bt in range(4):
                        r0 = ch * NB + bt * 128
                        nc.sync.dma_start(
                            xt_all[:, :, bt * 128:(bt + 1) * 128],
                            xs[r0:r0 + 128, :, :], transpose=True)
                    xts = [xt_all[:, 2 * j + 1, :] for j in range(16)]
                    if lvl < 3:
                        continue
                else:
                    xbs = []
                    for bt in range(4):
                        xb = xb_pool.tile([128, D_IN], bf16, tag="xb")
                        r0 = ch * NB + bt * 128
                        nc.gpsimd.dma_start(xb[:], xs[r0:r0 + 128, :])
                        xbs.append(xb)
                    if lvl < 2:
                        continue
                    xts = []
                    for jp in range(8):
                        pt = psT.tile([128, 2 * NB], bf16, tag="psT")
                        for jl in range(2):
                            j = 2 * jp + jl
                            for bt in range(4):
                                nc.tensor.transpose(
                                    pt[:, jl * NB + bt * 128:
                                       jl * NB + (bt + 1) * 128],
                                    xbs[bt][:, j * 128:(j + 1) * 128],
                                    eye128_sb[:])
                        xt = xt_pool.tile([128, 2, NB], bf16, tag="xt")
                        nc.vector.tensor_copy(xt[:], pt[:])
                        xts.extend([xt[:, 0, :], xt[:, 1, :]])
                    if lvl < 3:
                        continue

                featT = []
                for ft in range(4):
                    ps = psA.tile([128, NB], f32, tag="psA")
                    for j in range(16):
                        nc.tensor.matmul(
                            ps[:], wenc_sb[:, j, ft * 128:(ft + 1) * 128],
                            xts[j], start=(j == 0), stop=(j == 15),
                        )
                    fT = feat_pool.tile([128, NB], bf16, tag="featT")
                    nc.vector.tensor_scalar_add(fT[:], ps[:],
                                                benc_sb[:, ft:ft + 1])
                    featT.append(fT)
                if lvl < 4:
                    continue

                qfT = []
                for ft in range(4):
                    ps = psA.tile([128, NB], f32, tag="psA")
                    for k in range(4):
                        nc.tensor.matmul(
                            ps[:], wq_sb[:, k, ft * 128:(ft + 1) * 128],
                            featT[k][:], start=(k == 0), stop=(k == 3),
                        )
                    qT = qf_pool.tile([128, NB], bf16, tag="qfT")
                    nc.vector.tensor_scalar_add(qT[:], ps[:],
                                                bq_sb[:, ft:ft + 1])
                    qfT.append(qT)
                if lvl < 5:
                    continue

                # attention, row-wise: scores[b, m] -> exp+rowsum ->
                # normalize -> PE-transpose -> value matmul
                mos = []
                for h in range(H):
                    p0 = 64 * (h % 2)
                    ats = []
                    for bt in range(4):
                        ps = psS.tile([128, M], f32, tag="psS")
                        nc.tensor.matmul(
                            ps[:],
                            qfT[h // 2][p0:p0 + 64, bt * 128:(bt + 1) * 128],
                            mkt_bf[:, h // 2, :][p0:p0 + 64, :],
                            start=True, stop=True,
                        )
                        u = u_pool.tile([128, M], bf16, tag="u")
                        ssum = rs_pool.tile([128, 1], f32, tag="scol")
                        nc.scalar.activation(u[:], ps[:], FT.Exp,
                                             scale=INV_SQRT_DH,
                                             accum_out=ssum[:])
                        rcol = rs_pool.tile([128, 1], f32, tag="scol")
                        nc.vector.reciprocal(rcol[:], ssum[:])
                        at = u_pool.tile([128, M], bf16, tag="at")
                        nc.vector.tensor_scalar_mul(at[:], u[:], rcol[:])
                        ats.append(at)
                    pt = psT.tile([128, 2 * NB], bf16, tag="psT")
                    for bt in range(4):
                        for mj in range(2):
                            nc.tensor.transpose(
                                pt[:, mj * NB + bt * 128:
                                   mj * NB + (bt + 1) * 128],
                                ats[bt][:, mj * 128:(mj + 1) * 128],
                                eye128_sb[:])
                    att = nm_pool.tile([128, 2, NB], bf16, tag="att")
                    if h % 2 == 0:
                        nc.scalar.copy(att[:], pt[:])
                    else:
                        nc.vector.tensor_copy(att[:], pt[:])
                    if h % 2 == 0:
                        pm = psA.tile([128, NB], f32, tag="psA")
                        pm_cur = pm
                    p0o = DH * (h % 2)
                    for mj in range(2):
                        nc.tensor.matmul(
                            pm_cur[p0o:p0o + DH, :],
                            mv_nat[:, mj, h * DH:(h + 1) * DH],
                            att[:, mj, :], start=(mj == 0), stop=(mj == 1),
                        )
                    if h % 2 == 1:
                        mo = mo_pool.tile([128, NB], bf16, tag="mo")
                        nc.scalar.copy(mo[:], pm_cur[:])
                        mos.append(mo)
                if lvl < 6:
                    continue

                # classifier: logits^T[c, b], then transpose + store
                ps = psA.tile([128, NB], f32, tag="psA")
                for k in range(4):
                    nc.tensor.matmul(
                        ps[0:NW, :], wclst_sb[:, k, :], featT[k][:],
                        start=(k == 0), stop=False,
                    )
                for hp in range(4):
                    nc.tensor.matmul(
                        ps[0:NW, :], wclsb_sb[:, hp, :], mos[hp][:],
                        start=False, stop=(hp == 3),
                    )
                lg = lg_pool.tile([NW, NB], f32, tag="lg")
                nc.scalar.activation(lg[:], ps[0:NW, :], FT.Identity,
                                     bias=bcls_sb[:])
                po = psL.tile([128, 4 * NW], f32, tag="psL")
                for jb in range(4):
                    nc.tensor.transpose(
                        po[:, jb * NW:(jb + 1) * NW],
                        lg[:, jb * 128:(jb + 1) * 128], eye5_sb[:])
                ysb = y_pool.tile([128, 4 * NW], f32, tag="ysb")
                nc.vector.tensor_copy(ysb[:], po[:])
                nc.sync.dma_start(
                    y[ch * NB:(ch + 1) * NB, :].rearrange(
                        "(jb p) c -> p jb c", p=128),
                    ysb[:].rearrange("p (jb c) -> p jb c", c=NW))

            if stage != "full":
                # truncated build: still produce the output tensor
                for ch in range(NCH):
                    ysb = y_pool.tile([128, 4 * NW], f32, tag="ysb")
                    nc.vector.memset(ysb[:], 0.0)
                    nc.sync.dma_start(
                        y[ch * NB:(ch + 1) * NB, :].rearrange(
                            "(jb p) c -> p jb c", p=128),
                        ysb[:].rearrange("p (jb c) -> p jb c", c=NW))

    nc.compile()
    return nc


def prep_inputs(inputs):
    """Host-side shard/layout prep. Returns per-core in_maps."""
    x = np.ascontiguousarray(np.asarray(inputs["x"], dtype=np.float32))
    sx = np.asarray(inputs["support_x"], dtype=np.float32)
    sy = np.asarray(inputs["support_y"]).astype(np.int64)
    W_enc = np.asarray(inputs["W_enc"], dtype=np.float32)
    b_enc = np.asarray(inputs["b_enc"], dtype=np.float32)
    W_q = np.asarray(inputs["W_q"], dtype=np.float32)
    b_q = np.asarray(inputs["b_q"], dtype=np.float32)
    W_cls = np.asarray(inputs["W_cls"], dtype=np.float32)
    b_cls = np.asarray(inputs["b_cls"], dtype=np.float32)
    mem_keys = np.asarray(inputs["mem_keys"], dtype=np.float32)
    mem_values = np.asarray(inputs["mem_values"], dtype=np.float32)

    def pk(a, p=128):  # [K, N] -> [p, K/p, N] partition-major tiles
        k, n = a.shape
        return np.ascontiguousarray(a.reshape(k // p, p, n).transpose(1, 0, 2))

    if XT_MODE == "dmat":
        # row-permuted tiles: partition p of k-tile holds input-dim p*16+k,
        # matching the xbar-transpose output interleave
        wenc_h = np.ascontiguousarray(_bf(W_enc).reshape(128, 16, F))
        sxt_h = np.ascontiguousarray(_bf(sx.T).reshape(128, 16, NS))
    else:
        wenc_h = pk(_bf(W_enc))                  # [128, 16, F]
        sxt_h = pk(_bf(sx.T))                    # [128, 16, NS]
    wq_h = pk(_bf(W_q))                          # [128, 4, F]
    wclst_h = pk(_bf(W_cls[:F]))                 # [128, 4, NW]
    wclsb_h = pk(_bf(W_cls[F:]))                 # [128, 4, NW] head-pairs
    mkt = np.ascontiguousarray(mem_keys.T)       # [F, M]
    mkt0_h = pk(mkt)
    mkt0b_h = pk(_bf(mkt * (512.0 ** -0.5)))
    mvals_h = pk(_bf(mem_values))                # [128, 2, F]
    vals = np.zeros((NS, NW), np.float32)
    vals[np.arange(NS), sy] = 1.0
    valsb_h = pk(_bf(vals))                      # [128, 4, NW]
    benc_h = np.ascontiguousarray(b_enc.reshape(4, 128).T)
    bq_h = np.ascontiguousarray(b_q.reshape(4, 128).T)
    bcls_h = np.ascontiguousarray(b_cls.reshape(NW, 1))

    shared = dict(
        wenc=wenc_h, sxt=sxt_h, wq=wq_h, wclst=wclst_h, wclsb=wclsb_h,
        mkt0=mkt0_h, mkt0b=mkt0b_h, mvals=mvals_h, valsb=valsb_h,
        benc=benc_h, bq=bq_h, bcls=bcls_h,
    )
    in_maps = []
    for c in range(NCORES):
        m = dict(shared)
        if XT_MODE == "dmat":
            m["xsb"] = x[c * BC:(c + 1) * BC].view(
                ml_dtypes.bfloat16).reshape(BC, D_IN, 2)
        else:
            m["xs"] = x[c * BC:(c + 1) * BC]
        in_maps.append(m)
    return in_maps


def kernel_ex(inputs, trace=False, **kwargs):
    nc = build()
    in_maps = prep_inputs(inputs)
    res = run_bass_kernel_spmd(nc, in_maps, core_ids=list(range(NCORES)),
                               trace=trace, **kwargs)
    out = np.concatenate([r["y"] for r in res.results], axis=0)
    return out.astype(np.float32), res


def kernel(**inputs):
    out, _ = kernel_ex(inputs)
    return out
